# revision 1
# baseline (speedup 1.0000x reference)
"""Multi-head causal self-attention on 8 Trainium2 NeuronCores.

Problem: B=4, S=2048, D=1024, H=16 heads (Dh=64), fp32, causal + key-padding
mask, out = softmax(mask(QK^T/sqrt(Dh))) V Wo^T with Q/K/V = x @ W*^T.

Sharding (data-parallel over batch x tensor-parallel over heads):
  core = 2*b + g  (b in 0..3, g in 0..1): batch b, head group g (8 heads).
  Each core computes its 8 heads' attention and a partial output projection
  through its row-slice of Wo; the host sums the two partials per batch.

Schedule (v5): fully chunk-interleaved.
  - proj(0) head; then for each q-chunk c: attention tile-groups of chunk c
    interleaved with "filler" granules = proj(c+1) + wo(c-1); wo(3) tail.
  - Head PAIRS (2f, 2f+1) in rows 0-63/64-127 of feature tile f issue their
    score matmuls back-to-back at tile_position (0,0)/(64,0): the PE runs
    row-disjoint quadrant matmuls concurrently (~110ns vs 520ns per 512-wide
    64-contract matmul, measured).
  - Both heads' scores land in one [P, 2*QCH] psum pair-tile -> a single
    fused exp (Act) and a single fused tri-mask (DVE) per k-tile.
  - AV matmuls trail the scores by one k-tile so the PE never waits on exp.
  - V carries an appended ones-column per head so AV also yields the softmax
    denominators (row 64 of the [65, q] psum tile).
"""

import os
import numpy as np

import concourse.bass as bass
import concourse.mybir as mybir
import concourse.tile as tile
from concourse import bacc
from concourse.bass_utils import run_bass_kernel_spmd

P = 128
NEG = -1.0e30


def _round_f32r(a: np.ndarray) -> np.ndarray:
    bits = np.ascontiguousarray(a, dtype=np.float32).view(np.uint32)
    low = bits & np.uint32(0xFFF)
    hi = bits & np.uint32(0xFFFFF000)
    add = (low > 0x800) | ((low == 0x800) & (((bits >> 12) & 1) == 1))
    return (hi + (add.astype(np.uint32) << 12)).view(np.float32)


class Cfg:
    def __init__(self, B=4, S=2048, D=1024, H=16, Dh=64, n_cores=8, qch=512,
                 mm_dtype="bf16", reps=1):
        self.reps = reps
        self.B, self.S, self.D, self.H, self.Dh = B, S, D, H, Dh
        self.n_cores = n_cores
        self.groups = n_cores // B
        self.Hc = H // self.groups
        self.F = self.Hc * Dh
        self.qch = qch
        self.nqc = S // qch
        self.qt_per_ch = qch // P
        self.nt_s = S // P
        self.nt_d = D // P
        self.nt_f = self.F // P
        self.heads_per_ft = P // Dh
        self.mm_dtype = mm_dtype

    @property
    def mdt(self):
        return {"fp32r": mybir.dt.float32r,
                "fp32": mybir.dt.float32,
                "bf16": mybir.dt.bfloat16}[self.mm_dtype]


def build_nc(cfg: Cfg):
    f32 = mybir.dt.float32
    mdt = cfg.mdt
    S, D, F, Dh = cfg.S, cfg.D, cfg.F, cfg.Dh
    QCH = cfg.qch
    assert QCH == 512

    nc = bacc.Bacc("TRN2", target_bir_lowering=False, debug=False,
                   num_devices=cfg.n_cores)

    xT = nc.dram_tensor("xT", [D, S], mdt, kind="ExternalInput").ap()
    wqT = nc.dram_tensor("wqT", [D, F], mdt, kind="ExternalInput").ap()
    wkT = nc.dram_tensor("wkT", [D, F], mdt, kind="ExternalInput").ap()
    wvT = nc.dram_tensor("wvT", [D, F], mdt, kind="ExternalInput").ap()
    woT = nc.dram_tensor("woT", [F, D], mdt, kind="ExternalInput").ap()
    pbias = nc.dram_tensor("pbias", [P, cfg.nt_s], f32, kind="ExternalInput").ap()
    out = nc.dram_tensor("out", [S, D], f32, kind="ExternalOutput").ap()

    Exp = mybir.ActivationFunctionType.Exp
    mult = mybir.AluOpType.mult

    with tile.TileContext(nc) as tc:
        with (
            tc.tile_pool(name="psA", bufs=2, space="PSUM") as psA,   # [P,2*QCH]
            tc.tile_pool(name="psB", bufs=2, space="PSUM") as psB,   # pav
            tc.tile_pool(name="psC", bufs=2, space="PSUM") as psC,   # proj/wo
            tc.tile_pool(name="sb_kT", bufs=cfg.nt_f) as sb_kT,
            tc.tile_pool(name="sb_v", bufs=cfg.nt_s) as sb_v,
            tc.tile_pool(name="sb_misc", bufs=1) as sb_misc,
        ):
            # --- constants ---
            pb = sb_misc.tile([P, cfg.nt_s], f32, tag="pbias")
            nc.sync.dma_start(pb[:], pbias)
            tri_f = sb_misc.tile([P, P], f32, tag="tri_f")
            nc.gpsimd.memset(tri_f[:], 1.0)
            nc.gpsimd.affine_select(
                out=tri_f[:], in_=tri_f[:],
                compare_op=mybir.AluOpType.is_ge, fill=0.0,
                base=0, channel_multiplier=-1, pattern=[[1, P]],
            )
            tri = sb_misc.tile([P, P], mdt, tag="tri")
            nc.vector.tensor_copy(tri[:], tri_f[:])
            ones_c = sb_misc.tile([P, 1], f32, tag="ones_c")
            nc.gpsimd.memset(ones_c[:], 1.0)

            kT_t = [sb_kT.tile([P, S], mdt, tag="kT", name="kT")
                    for _ in range(cfg.nt_f)]
            v_t = [sb_v.tile([P, cfg.Hc * (Dh + 1)], mdt, tag="v", name="v")
                   for _ in range(cfg.nt_s)]

            for _rep in range(cfg.reps):
              with (
                tc.tile_pool(name=f"sb_xt{_rep}", bufs=cfg.nt_d) as sb_xt,
                tc.tile_pool(name=f"sb_w{_rep}", bufs=3 * cfg.nt_d) as sb_w,
                tc.tile_pool(name=f"sb_wo{_rep}", bufs=cfg.nt_f) as sb_wo,
                tc.tile_pool(name=f"sb_qT{_rep}", bufs=2 * cfg.nt_f) as sb_qT,
                tc.tile_pool(name=f"sb_ctx{_rep}", bufs=4 * cfg.nt_f) as sb_ctx,
                tc.tile_pool(name=f"sb_exp{_rep}", bufs=8) as sb_exp,
                tc.tile_pool(name=f"sb_out{_rep}", bufs=3) as sb_out,
                tc.tile_pool(name=f"sb_rc{_rep}", bufs=4) as sb_rc,
                tc.tile_pool(name=f"sb_sn{_rep}", bufs=4) as sb_sn,
              ):
                def _wload(wdram, n=None, pool=None, width=None):
                    lst = []
                    for d in range(n or cfg.nt_d):
                        t = (pool or sb_w).tile([P, width or F], mdt, tag="w",
                                                name="w")
                        nc.sync.dma_start(t[:], wdram[d * P:(d + 1) * P, :])
                        lst.append(t)
                    return lst

                def _xload(c):
                    lst = []
                    for d in range(cfg.nt_d):
                        t = sb_xt.tile([P, QCH], mdt, tag="xt", name="xt")
                        nc.sync.dma_start(
                            t[:], xT[d * P:(d + 1) * P, c * QCH:(c + 1) * QCH])
                        lst.append(t)
                    return lst

                wq_t = _wload(wqT)
                xt = {0: _xload(0)}
                wk_t = _wload(wkT)
                wv_t = _wload(wvT)
                wo_t = _wload(woT, n=cfg.nt_f, pool=sb_wo, width=D)

                qT = {}    # c -> [nt_f tiles of [P, QCH]]
                ctx = {}   # c -> [nt_f tiles of [P, QCH]]

                def _proj_granules(c):
                    """Return a list of emitter thunks for chunk c's q/k/v."""
                    qT[c] = [sb_qT.tile([P, QCH], mdt, tag="qT", name="qT")
                             for _ in range(cfg.nt_f)]
                    gs = []

                    def _qk(wt, m, dst_tile, dst_cols):
                        def g():
                            ps = psC.tile([P, QCH], f32, tag="psC", name="ps")
                            for d in range(cfg.nt_d):
                                nc.tensor.matmul(
                                    ps[:],
                                    wt[d][:, m * P:(m + 1) * P],
                                    xt[c][d][:],
                                    start=(d == 0), stop=(d == cfg.nt_d - 1),
                                )
                            nc.vector.tensor_copy(dst_tile[:, dst_cols], ps[:])
                        return g

                    def _v(u):
                        def g():
                            st = c * cfg.qt_per_ch + u
                            ps = psC.tile([P, F], f32, tag="psC", name="ps")
                            for d in range(cfg.nt_d):
                                nc.tensor.matmul(
                                    ps[:],
                                    xt[c][d][:, u * P:(u + 1) * P],
                                    wv_t[d][:],
                                    start=(d == 0), stop=(d == cfg.nt_d - 1),
                                )
                            dst = v_t[st][:].rearrange("p (h e) -> p h e",
                                                       e=Dh + 1)
                            nc.vector.tensor_copy(
                                dst[:, :, 0:Dh],
                                ps[:].rearrange("p (h e) -> p h e", e=Dh),
                            )
                            nc.vector.tensor_copy(
                                dst[:, :, Dh:Dh + 1],
                                ones_c[:, None, 0:1].to_broadcast([P, cfg.Hc, 1]))
                        return g

                    # k granules last: the kT tile write WARs (tile-granular)
                    # on pair m's scores still reading kT_t[m] this chunk, so
                    # they should fire as late as possible.
                    for m in range(cfg.nt_f):
                        gs.append(_qk(wq_t, m, qT[c][m], slice(0, QCH)))
                    for u in range(cfg.qt_per_ch):
                        gs.append(_v(u))
                    for m in range(cfg.nt_f):
                        gs.append(_qk(wk_t, m, kT_t[m],
                                      slice(c * QCH, (c + 1) * QCH)))
                    return gs

                def _wo_granule(c, u):
                    def g():
                        st = c * cfg.qt_per_ch + u
                        ot = sb_out.tile([P, D], f32, tag="ot", name="ot")
                        for dch in range(D // 512):
                            pwo = psC.tile([P, 512], f32, tag="psC", name="pwo")
                            for f2 in range(cfg.nt_f):
                                nc.tensor.matmul(
                                    pwo[:],
                                    ctx[c][f2][:, u * P:(u + 1) * P],
                                    wo_t[f2][:, dch * 512:(dch + 1) * 512],
                                    start=(f2 == 0), stop=(f2 == cfg.nt_f - 1),
                                )
                            nc.vector.tensor_copy(
                                ot[:, dch * 512:(dch + 1) * 512], pwo[:])
                        nc.sync.dma_start(out[st * P:(st + 1) * P, :], ot[:])
                    return g

                def _attn_pair(c, f, fillers, fill_state):
                    """Attention for head pair (2f, 2f+1), chunk c; pops
                    filler thunks at a steady rate between tile-groups."""
                    ktiles = cfg.qt_per_ch * (c + 1)
                    rA, rB = slice(0, Dh), slice(Dh, 2 * Dh)
                    hA, hB = 2 * f, 2 * f + 1
                    pavA = psB.tile([Dh + 1, QCH], f32, tag="pav")
                    pavB = psB.tile([Dh + 1, QCH], f32, tag="pav")
                    pending = []
                    for t in range(ktiles):
                        j = t - cfg.qt_per_ch * c
                        col0 = max(0, j * P)
                        pss = psA.tile([P, 2 * QCH], f32, tag="psA", name="pss")
                        nc.tensor.matmul(
                            pss[:, col0:QCH],
                            kT_t[f][rA, t * P:(t + 1) * P],
                            qT[c][f][rA, col0:QCH],
                            start=True, stop=True, tile_position=(0, 0),
                        )
                        nc.tensor.matmul(
                            pss[:, QCH + col0:2 * QCH],
                            kT_t[f][rB, t * P:(t + 1) * P],
                            qT[c][f][rB, col0:QCH],
                            start=True, stop=True, tile_position=(Dh, 0),
                        )
                        etP = sb_exp.tile([P, 2, QCH], mdt, tag="exp")
                        pss2 = pss[:].rearrange("p (h w) -> p h w", h=2)
                        nc.scalar.activation(
                            etP[:, :, col0:], pss2[:, :, col0:], Exp,
                            bias=pb[:, t:t + 1], scale=float(Dh) ** -0.5,
                        )
                        if j >= 0:
                            nc.vector.tensor_tensor(
                                etP[:, :, col0:col0 + P],
                                etP[:, :, col0:col0 + P],
                                tri[:, None, :].to_broadcast([P, 2, P]), mult)
                        pending.append((t, col0, etP))
                        if len(pending) > 3:
                            pt, pcol0, petP = pending.pop(0)
                            nc.tensor.matmul(
                                pavA[:, pcol0:],
                                v_t[pt][:, hA * (Dh + 1):(hA + 1) * (Dh + 1)],
                                petP[:, 0, pcol0:],
                                start=(pt == 0), stop=False,
                            )
                            nc.tensor.matmul(
                                pavB[:, pcol0:],
                                v_t[pt][:, hB * (Dh + 1):(hB + 1) * (Dh + 1)],
                                petP[:, 1, pcol0:],
                                start=(pt == 0), stop=False,
                            )
                        # steady-rate filler emission
                        fill_state[0] += fill_state[1]
                        while fill_state[0] >= 1.0 and fillers:
                            fill_state[0] -= 1.0
                            fillers.pop(0)()
                    while pending:
                        pt, pcol0, petP = pending.pop(0)
                        last = not pending
                        nc.tensor.matmul(
                            pavA[:, pcol0:],
                            v_t[pt][:, hA * (Dh + 1):(hA + 1) * (Dh + 1)],
                            petP[:, 0, pcol0:],
                            start=(pt == 0), stop=last,
                        )
                        nc.tensor.matmul(
                            pavB[:, pcol0:],
                            v_t[pt][:, hB * (Dh + 1):(hB + 1) * (Dh + 1)],
                            petP[:, 1, pcol0:],
                            start=(pt == 0), stop=last,
                        )
                    for pav, rows in ((pavA, rA), (pavB, rB)):
                        sn = sb_sn.tile([Dh + 1, QCH], f32, tag="sn")
                        nc.vector.tensor_copy(sn[:], pav[:])
                        rc = sb_rc.tile([1, QCH], f32, tag="rc")
                        rcb = sb_rc.tile([Dh, QCH], f32, tag="rcb")
                        nc.vector.reciprocal(rc[:], sn[Dh:Dh + 1, :])
                        nc.gpsimd.partition_broadcast(rcb[:], rc[:])
                        nc.vector.tensor_tensor(
                            ctx[c][f][rows, :], sn[0:Dh, :], rcb[:], mult)

                # ---- head: chunk 0 projections, un-overlapped ----
                for g in _proj_granules(0):
                    g()

                for c in range(cfg.nqc):
                    ctx[c] = [sb_ctx.tile([P, QCH], mdt, tag="ctx", name="ctx")
                              for _ in range(cfg.nt_f)]
                    fillers = []
                    if c + 1 < cfg.nqc:
                        xt[c + 1] = _xload(c + 1)
                        fillers += _proj_granules(c + 1)
                    if c == cfg.nqc - 1:
                        for cc in range(cfg.nqc - 1):
                            fillers += [_wo_granule(cc, u)
                                        for u in range(cfg.qt_per_ch)]
                    n_groups = cfg.qt_per_ch * (c + 1) * cfg.nt_f
                    fill_state = [0.0, len(fillers) / n_groups]
                    for f in range(cfg.nt_f):
                        _attn_pair(c, f, fillers, fill_state)
                    for g in fillers:  # leftovers (rounding)
                        g()
                for u in range(cfg.qt_per_ch):
                    _wo_granule(cfg.nqc - 1, u)()

    nc.compile()
    return nc


_NC_CACHE = {}


def _get_nc(cfg: Cfg):
    key = (cfg.B, cfg.S, cfg.D, cfg.H, cfg.n_cores, cfg.qch, cfg.mm_dtype,
           cfg.reps)
    if key not in _NC_CACHE:
        _NC_CACHE[key] = build_nc(cfg)
    return _NC_CACHE[key]


def make_in_maps(cfg: Cfg, x_self, padding_mask, Wq, Wk, Wv, Wo):
    if cfg.mm_dtype == "fp32r":
        rnd = _round_f32r
    elif cfg.mm_dtype == "bf16":
        import ml_dtypes
        rnd = lambda a: np.ascontiguousarray(np.asarray(a, dtype=np.float32)).astype(ml_dtypes.bfloat16)
    else:
        rnd = lambda a: np.ascontiguousarray(a, dtype=np.float32)
    in_maps = []
    for core in range(cfg.n_cores):
        b, g = divmod(core, cfg.groups)
        fsl = slice(g * cfg.F, (g + 1) * cfg.F)
        pbias = np.where(padding_mask[b], np.float32(NEG), np.float32(0.0))
        in_maps.append({
            "xT": rnd(x_self[b].T),
            "wqT": rnd(Wq[fsl, :].T),
            "wkT": rnd(Wk[fsl, :].T),
            "wvT": rnd(Wv[fsl, :].T),
            "woT": rnd(Wo[:, fsl].T),
            "pbias": np.ascontiguousarray(
                pbias.reshape(cfg.nt_s, P).T).astype(np.float32),
        })
    return in_maps


def kernel(x_self, x_other, padding_mask, Wq, Wk, Wv, Wo, _trace=False):
    x_self = np.asarray(x_self, dtype=np.float32)
    padding_mask = np.asarray(padding_mask)
    Wq = np.asarray(Wq, dtype=np.float32)
    Wk = np.asarray(Wk, dtype=np.float32)
    Wv = np.asarray(Wv, dtype=np.float32)
    Wo = np.asarray(Wo, dtype=np.float32)

    B, S, D = x_self.shape
    cfg = Cfg(B=B, S=S, D=D)
    nc = _get_nc(cfg)
    in_maps = make_in_maps(cfg, x_self, padding_mask, Wq, Wk, Wv, Wo)
    res = run_bass_kernel_spmd(
        nc, in_maps, core_ids=list(range(cfg.n_cores)), trace=_trace)

    out = np.zeros((B, S, D), dtype=np.float32)
    for core in range(cfg.n_cores):
        b = core // cfg.groups
        out[b] += res.results[core]["out"]
    if _trace:
        kernel.last_exec_time_ns = res.exec_time_ns
        kernel.last_results = res
    return out



# revision 20
# speedup vs baseline: 7.1971x; 7.1971x over previous
"""Multi-head causal self-attention on 8 Trainium2 NeuronCores.

Problem: B=4, S=2048, D=1024, H=16 heads (Dh=64), fp32, causal + key-padding
mask, out = softmax(mask(QK^T/sqrt(Dh))) V Wo^T with Q/K/V = x @ W*^T.

Sharding (data-parallel over batch x tensor-parallel over heads):
  core = 2*b + g  (b in 0..3, g in 0..1): batch b, head group g (8 heads).
  Each core computes its 8 heads' attention and a partial output projection
  through its row-slice of Wo; the host sums the two partials per batch.

Schedule (v5): fully chunk-interleaved.
  - proj(0) head; then for each q-chunk c: attention tile-groups of chunk c
    interleaved with "filler" granules = proj(c+1) + wo(c-1); wo(3) tail.
  - Head PAIRS (2f, 2f+1) in rows 0-63/64-127 of feature tile f issue their
    score matmuls back-to-back at tile_position (0,0)/(64,0): the PE runs
    row-disjoint quadrant matmuls concurrently (~110ns vs 520ns per 512-wide
    64-contract matmul, measured).
  - Both heads' scores land in one [P, 2*QCH] psum pair-tile -> a single
    fused exp (Act) and a single fused tri-mask (DVE) per k-tile.
  - AV matmuls trail the scores by one k-tile so the PE never waits on exp.
  - V carries an appended ones-column per head so AV also yields the softmax
    denominators (row 64 of the [65, q] psum tile).
"""

import os
import numpy as np

import concourse.bass as bass
import concourse.mybir as mybir
import concourse.tile as tile
from concourse import bacc
from concourse.bass_utils import run_bass_kernel_spmd

P = 128
NEG = -1.0e30


def _round_f32r(a: np.ndarray) -> np.ndarray:
    bits = np.ascontiguousarray(a, dtype=np.float32).view(np.uint32)
    low = bits & np.uint32(0xFFF)
    hi = bits & np.uint32(0xFFFFF000)
    add = (low > 0x800) | ((low == 0x800) & (((bits >> 12) & 1) == 1))
    return (hi + (add.astype(np.uint32) << 12)).view(np.float32)


class Cfg:
    def __init__(self, B=4, S=2048, D=1024, H=16, Dh=64, n_cores=8, qch=512,
                 mm_dtype="bf16", reps=1):
        self.reps = reps
        self.B, self.S, self.D, self.H, self.Dh = B, S, D, H, Dh
        self.n_cores = n_cores
        self.groups = n_cores // B
        self.Hc = H // self.groups
        self.F = self.Hc * Dh
        self.qch = qch
        self.nqc = S // qch
        self.qt_per_ch = qch // P
        self.nt_s = S // P
        self.nt_d = D // P
        self.nt_f = self.F // P
        self.heads_per_ft = P // Dh
        self.mm_dtype = mm_dtype

    @property
    def mdt(self):
        return {"fp32r": mybir.dt.float32r,
                "fp32": mybir.dt.float32,
                "bf16": mybir.dt.bfloat16}[self.mm_dtype]


def build_nc(cfg: Cfg):
    f32 = mybir.dt.float32
    mdt = cfg.mdt
    S, D, F, Dh = cfg.S, cfg.D, cfg.F, cfg.Dh
    QCH = cfg.qch
    assert QCH == 512

    nc = bacc.Bacc("TRN2", target_bir_lowering=False, debug=False,
                   num_devices=cfg.n_cores)

    xT = nc.dram_tensor("xT", [D, S], mdt, kind="ExternalInput").ap()
    wqT = nc.dram_tensor("wqT", [D, F], mdt, kind="ExternalInput").ap()
    wkT = nc.dram_tensor("wkT", [D, F], mdt, kind="ExternalInput").ap()
    wvT = nc.dram_tensor("wvT", [D, F], mdt, kind="ExternalInput").ap()
    woT = nc.dram_tensor("woT", [F, D], mdt, kind="ExternalInput").ap()
    pbias = nc.dram_tensor("pbias", [P, cfg.nt_s], f32, kind="ExternalInput").ap()
    out = nc.dram_tensor("out", [S, D], f32, kind="ExternalOutput").ap()

    Exp = mybir.ActivationFunctionType.Exp
    mult = mybir.AluOpType.mult

    with tile.TileContext(nc) as tc:
        with (
            tc.tile_pool(name="psA", bufs=2, space="PSUM") as psA,   # [P,2*QCH]
            tc.tile_pool(name="psB", bufs=2, space="PSUM") as psB,   # pav
            tc.tile_pool(name="psC", bufs=2, space="PSUM") as psC,   # proj/wo
            tc.tile_pool(name="sb_kT", bufs=cfg.nt_f) as sb_kT,
            tc.tile_pool(name="sb_v", bufs=cfg.nt_s) as sb_v,
            tc.tile_pool(name="sb_misc", bufs=1) as sb_misc,
        ):
            # --- constants ---
            pb = sb_misc.tile([P, cfg.nt_s], f32, tag="pbias")
            nc.sync.dma_start(pb[:], pbias)
            tri_f = sb_misc.tile([P, P], f32, tag="tri_f")
            nc.gpsimd.memset(tri_f[:], 1.0)
            nc.gpsimd.affine_select(
                out=tri_f[:], in_=tri_f[:],
                compare_op=mybir.AluOpType.is_ge, fill=0.0,
                base=0, channel_multiplier=-1, pattern=[[1, P]],
            )
            tri = sb_misc.tile([P, P], mdt, tag="tri")
            nc.vector.tensor_copy(tri[:], tri_f[:])
            ones_c = sb_misc.tile([P, 1], f32, tag="ones_c")
            nc.gpsimd.memset(ones_c[:], 1.0)
            # identity matrix: permutation rhs for PE transposes
            id_f = sb_misc.tile([P, P], f32, tag="id_f")
            nc.gpsimd.memset(id_f[:], 1.0)
            nc.gpsimd.affine_select(
                out=id_f[:], in_=id_f[:],
                compare_op=mybir.AluOpType.is_ge, fill=0.0,
                base=0, channel_multiplier=-1, pattern=[[1, P]],
            )
            nc.gpsimd.affine_select(
                out=id_f[:], in_=id_f[:],
                compare_op=mybir.AluOpType.is_ge, fill=0.0,
                base=0, channel_multiplier=1, pattern=[[-1, P]],
            )
            id_t = sb_misc.tile([P, P], mdt, tag="id_t")
            nc.vector.tensor_copy(id_t[:], id_f[:])

            kT_t = [sb_kT.tile([P, S], mdt, tag="kT", name="kT")
                    for _ in range(cfg.nt_f)]
            v_t = [sb_v.tile([P, cfg.Hc * (Dh + 1)], mdt, tag="v", name="v")
                   for _ in range(cfg.nt_s)]

            for _rep in range(cfg.reps):
              with (
                tc.tile_pool(name=f"sb_xt{_rep}", bufs=cfg.nt_d) as sb_xt,
                tc.tile_pool(name=f"sb_w{_rep}", bufs=3 * cfg.nt_d) as sb_w,
                tc.tile_pool(name=f"sb_wo{_rep}", bufs=cfg.nt_f) as sb_wo,
                tc.tile_pool(name=f"sb_qT{_rep}", bufs=2 * cfg.nt_f) as sb_qT,
                tc.tile_pool(name=f"sb_ctx{_rep}", bufs=4 * cfg.nt_f) as sb_ctx,
                tc.tile_pool(name=f"sb_exp{_rep}", bufs=cfg.nt_s) as sb_exp,
                tc.tile_pool(name=f"sb_out{_rep}", bufs=3) as sb_out,
                tc.tile_pool(name=f"sb_rc{_rep}", bufs=4) as sb_rc,
                tc.tile_pool(name=f"sb_sn{_rep}", bufs=4) as sb_sn,
              ):
                def _wload(wdram, n=None, pool=None, width=None):
                    lst = []
                    for d in range(n or cfg.nt_d):
                        t = (pool or sb_w).tile([P, width or F], mdt, tag="w",
                                                name="w")
                        nc.sync.dma_start(t[:], wdram[d * P:(d + 1) * P, :])
                        lst.append(t)
                    return lst

                def _xload(c):
                    lst = []
                    for d in range(cfg.nt_d):
                        t = sb_xt.tile([P, QCH], mdt, tag="xt", name="xt")
                        nc.sync.dma_start(
                            t[:], xT[d * P:(d + 1) * P, c * QCH:(c + 1) * QCH])
                        lst.append(t)
                    return lst

                wq_t = _wload(wqT)
                xt = {0: _xload(0)}
                wk_t = _wload(wkT)
                wv_t = _wload(wvT)
                wo_t = _wload(woT, n=cfg.nt_f, pool=sb_wo, width=D)

                qT = {}    # c -> [nt_f tiles of [P, QCH]]
                ctx = {}   # c -> [nt_f tiles of [P, QCH]]

                def _proj_granules(c):
                    """Return a list of emitter thunks for chunk c's q/k/v."""
                    qT[c] = [sb_qT.tile([P, QCH], mdt, tag="qT", name="qT")
                             for _ in range(cfg.nt_f)]
                    gs = []

                    def _qk(wt, m, dst_tile, dst_cols):
                        def g():
                            ps = psC.tile([P, QCH], f32, tag="psC", name="ps")
                            for d in range(cfg.nt_d):
                                nc.tensor.matmul(
                                    ps[:],
                                    wt[d][:, m * P:(m + 1) * P],
                                    xt[c][d][:],
                                    start=(d == 0), stop=(d == cfg.nt_d - 1),
                                )
                            nc.vector.tensor_copy(dst_tile[:, dst_cols], ps[:])
                        return g

                    def _v(u):
                        def g():
                            st = c * cfg.qt_per_ch + u
                            ps = psC.tile([P, F], f32, tag="psC", name="ps")
                            for d in range(cfg.nt_d):
                                nc.tensor.matmul(
                                    ps[:],
                                    xt[c][d][:, u * P:(u + 1) * P],
                                    wv_t[d][:],
                                    start=(d == 0), stop=(d == cfg.nt_d - 1),
                                )
                            dst = v_t[st][:].rearrange("p (h e) -> p h e",
                                                       e=Dh + 1)
                            nc.vector.tensor_copy(
                                dst[:, :, 0:Dh],
                                ps[:].rearrange("p (h e) -> p h e", e=Dh),
                            )
                            nc.vector.tensor_copy(
                                dst[:, :, Dh:Dh + 1],
                                ones_c[:, None, 0:1].to_broadcast([P, cfg.Hc, 1]))
                        return g

                    # k granules last: the kT tile write WARs (tile-granular)
                    # on pair m's scores still reading kT_t[m] this chunk, so
                    # they should fire as late as possible.
                    for m in range(cfg.nt_f):
                        gs.append(_qk(wq_t, m, qT[c][m], slice(0, QCH)))
                    for u in range(cfg.qt_per_ch):
                        gs.append(_v(u))
                    for m in range(cfg.nt_f):
                        gs.append(_qk(wk_t, m, kT_t[m],
                                      slice(c * QCH, (c + 1) * QCH)))
                    return gs

                def _wo_granule(c, u):
                    def g():
                        st = c * cfg.qt_per_ch + u
                        ot = sb_out.tile([P, D], f32, tag="ot", name="ot")
                        for dch in range(D // 512):
                            pwo = psC.tile([P, 512], f32, tag="psC", name="pwo")
                            for f2 in range(cfg.nt_f):
                                nc.tensor.matmul(
                                    pwo[:],
                                    ctx[c][f2][:, u * P:(u + 1) * P],
                                    wo_t[f2][:, dch * 512:(dch + 1) * 512],
                                    start=(f2 == 0), stop=(f2 == cfg.nt_f - 1),
                                )
                            nc.vector.tensor_copy(
                                ot[:, dch * 512:(dch + 1) * 512], pwo[:])
                        nc.sync.dma_start(out[st * P:(st + 1) * P, :], ot[:])
                    return g

                def _attn_pair(c, f, fillers, fill_state):
                    """Attention for head pair (2f, 2f+1), chunk c; pops
                    filler thunks at a steady rate between tile-groups.
                    AV is TRANSPOSED: exp[k, q-block] is the stationary
                    operand, v+ones [k, 65] the moving one -> out
                    [q 128, 65] in 65 cycles/matmul (vs 128/q-block when
                    streaming exp), and the denominator (col 64) lands
                    per-partition so normalize is a cheap scalar mult.
                    ctx^T is PE-transposed back via the identity rhs."""
                    ktiles = cfg.qt_per_ch * (c + 1)
                    rA, rB = slice(0, Dh), slice(Dh, 2 * Dh)
                    hA, hB = 2 * f, 2 * f + 1
                    nu = cfg.qt_per_ch
                    # one psum bank per head holds the [q 128, u-slot,
                    # Dh+ones] accumulators. PSUM pending-zero is
                    # bank-granular on HW (a start=True re-arms
                    # zero-on-next-write for the WHOLE bank), so the four
                    # u-groups of a bank must run strictly sequentially:
                    # exp tiles are retained and AV runs as per-u sweeps
                    # after the score/exp stream.
                    bankA = psB.tile([P, QCH], f32, tag="pav")
                    bankB = psB.tile([P, QCH], f32, tag="pav")
                    pavA = bankA[:, 0:nu * (Dh + 1)].rearrange(
                        "p (u e) -> p u e", e=Dh + 1)
                    pavB = bankB[:, 0:nu * (Dh + 1)].rearrange(
                        "p (u e) -> p u e", e=Dh + 1)
                    etiles = []

                    for t in range(ktiles):
                        j = t - cfg.qt_per_ch * c
                        col0 = max(0, j * P)
                        pss = psA.tile([P, 2 * QCH], f32, tag="psA", name="pss")
                        nc.tensor.matmul(
                            pss[:, col0:QCH],
                            kT_t[f][rA, t * P:(t + 1) * P],
                            qT[c][f][rA, col0:QCH],
                            start=True, stop=True, tile_position=(0, 0),
                        )
                        nc.tensor.matmul(
                            pss[:, QCH + col0:2 * QCH],
                            kT_t[f][rB, t * P:(t + 1) * P],
                            qT[c][f][rB, col0:QCH],
                            start=True, stop=True, tile_position=(Dh, 0),
                        )
                        etP = sb_exp.tile([P, 2, QCH], mdt, tag="exp")
                        pss2 = pss[:].rearrange("p (h w) -> p h w", h=2)
                        nc.scalar.activation(
                            etP[:, :, col0:], pss2[:, :, col0:], Exp,
                            bias=pb[:, t:t + 1], scale=float(Dh) ** -0.5,
                        )
                        if j >= 0:
                            nc.vector.tensor_tensor(
                                etP[:, :, col0:col0 + P],
                                etP[:, :, col0:col0 + P],
                                tri[:, None, :].to_broadcast([P, 2, P]), mult)
                        etiles.append(etP)
                        # steady-rate filler emission
                        fill_state[0] += fill_state[1]
                        while fill_state[0] >= 1.0 and fillers:
                            fill_state[0] -= 1.0
                            fillers.pop(0)()
                    # AV sweeps: per u-slot, one clean start->stop run over
                    # its k-tiles (u=0 first: its exp tiles finished
                    # earliest, so the sweep never waits on the Act tail)
                    for u in range(nu):
                        tend = cfg.qt_per_ch * c + u
                        for t in range(tend + 1):
                            for pav, hs in ((pavA, hA * (Dh + 1)),
                                            (pavB, hB * (Dh + 1))):
                                h = 0 if pav is pavA else 1
                                nc.tensor.matmul(
                                    pav[:, u, :],
                                    etiles[t][:, h, u * P:(u + 1) * P],
                                    v_t[t][:, hs:hs + Dh + 1],
                                    start=(t == 0), stop=(t == tend),
                                )
                            fill_state[0] += 0.25
                            while fill_state[0] >= 1.0 and fillers:
                                fill_state[0] -= 1.0
                                fillers.pop(0)()
                    for bank, pav, rows in ((bankA, pavA, rA),
                                            (bankB, pavB, rB)):
                        rc = sb_rc.tile([P, nu, 1], f32, tag="rc")
                        nc.vector.reciprocal_approx_fast(
                            rc[:], pav[:, :, Dh:Dh + 1])
                        cT = sb_sn.tile([P, nu, Dh], f32, tag="cT")
                        nc.vector.tensor_tensor(
                            cT[:], pav[:, :, 0:Dh],
                            rc[:].to_broadcast([P, nu, Dh]), mult)
                        tpt = psC.tile([P, QCH], f32, tag="psC", name="tp")
                        tp = tpt[0:Dh, :].rearrange("e (u q) -> e u q", q=P)
                        for u in range(nu):
                            nc.tensor.matmul(
                                tp[:, u, :], cT[:, u, :], id_f[:],
                                start=True, stop=True, is_transpose=True,
                            )
                        nc.vector.tensor_copy(
                            ctx[c][f][rows, :],
                            tp[:].rearrange("e u q -> e (u q)"))

                # ---- head: chunk 0 projections, un-overlapped ----
                for g in _proj_granules(0):
                    g()

                for c in range(cfg.nqc):
                    ctx[c] = [sb_ctx.tile([P, QCH], mdt, tag="ctx", name="ctx")
                              for _ in range(cfg.nt_f)]
                    fillers = []
                    if c + 1 < cfg.nqc:
                        xt[c + 1] = _xload(c + 1)
                        fillers += _proj_granules(c + 1)
                    if c == cfg.nqc - 1:
                        for cc in range(cfg.nqc - 1):
                            fillers += [_wo_granule(cc, u)
                                        for u in range(cfg.qt_per_ch)]
                    n_groups = cfg.nt_f * (cfg.qt_per_ch * (c + 1)
                                           + 0.25 * (16 * c + 10))
                    fill_state = [0.0, len(fillers) / n_groups]
                    for f in range(cfg.nt_f):
                        _attn_pair(c, f, fillers, fill_state)
                    for g in fillers:  # leftovers (rounding)
                        g()
                for u in range(cfg.qt_per_ch):
                    _wo_granule(cfg.nqc - 1, u)()

    nc.compile()
    return nc


_NC_CACHE = {}


def _get_nc(cfg: Cfg):
    key = (cfg.B, cfg.S, cfg.D, cfg.H, cfg.n_cores, cfg.qch, cfg.mm_dtype,
           cfg.reps)
    if key not in _NC_CACHE:
        _NC_CACHE[key] = build_nc(cfg)
    return _NC_CACHE[key]


def make_in_maps(cfg: Cfg, x_self, padding_mask, Wq, Wk, Wv, Wo):
    if cfg.mm_dtype == "fp32r":
        rnd = _round_f32r
    elif cfg.mm_dtype == "bf16":
        import ml_dtypes
        rnd = lambda a: np.ascontiguousarray(np.asarray(a, dtype=np.float32)).astype(ml_dtypes.bfloat16)
    else:
        rnd = lambda a: np.ascontiguousarray(a, dtype=np.float32)
    in_maps = []
    for core in range(cfg.n_cores):
        b, g = divmod(core, cfg.groups)
        fsl = slice(g * cfg.F, (g + 1) * cfg.F)
        pbias = np.where(padding_mask[b], np.float32(NEG), np.float32(0.0))
        in_maps.append({
            "xT": rnd(x_self[b].T),
            "wqT": rnd(Wq[fsl, :].T),
            "wkT": rnd(Wk[fsl, :].T),
            "wvT": rnd(Wv[fsl, :].T),
            "woT": rnd(Wo[:, fsl].T),
            "pbias": np.ascontiguousarray(
                pbias.reshape(cfg.nt_s, P).T).astype(np.float32),
        })
    return in_maps


def kernel(x_self, x_other, padding_mask, Wq, Wk, Wv, Wo, _trace=False):
    x_self = np.asarray(x_self, dtype=np.float32)
    padding_mask = np.asarray(padding_mask)
    Wq = np.asarray(Wq, dtype=np.float32)
    Wk = np.asarray(Wk, dtype=np.float32)
    Wv = np.asarray(Wv, dtype=np.float32)
    Wo = np.asarray(Wo, dtype=np.float32)

    B, S, D = x_self.shape
    cfg = Cfg(B=B, S=S, D=D)
    nc = _get_nc(cfg)
    in_maps = make_in_maps(cfg, x_self, padding_mask, Wq, Wk, Wv, Wo)
    res = run_bass_kernel_spmd(
        nc, in_maps, core_ids=list(range(cfg.n_cores)), trace=_trace)

    out = np.zeros((B, S, D), dtype=np.float32)
    for core in range(cfg.n_cores):
        b = core // cfg.groups
        out[b] += res.results[core]["out"]
    if _trace:
        kernel.last_exec_time_ns = res.exec_time_ns
        kernel.last_results = res
    return out

